# revision 3
# baseline (speedup 1.0000x reference)
"""Causal self-attention (B=2, T=2048, C=1024, 16 heads x 64) on 8 NeuronCores.

Sharding: core = 4*b + hg  (b in {0,1} data-parallel over batch,
hg in {0..3} tensor-parallel over head groups of 4 heads).
Each core computes its 4 heads' attention and the partial c_proj
contribution y_heads @ w_proj[256hg:256hg+256]; the host sums the 4
partials per batch element (the "all-reduce after c_proj").

Device layout (per core):
  - x^T resident in SBUF as 8 chunks of [128, 2048] (C on partitions).
  - q^T,k^T [512, 2048] via PE with w slices stationary; two heads share
    each 128-partition chunk so S^T = K.T@... uses K=64 row-packed matmul
    pairs (tile_position rows 0-63 / 64-127 run concurrently).
  - V token-major [2048, 4*65] with a ones column per head, so the PV
    matmul (V|1 stationary, P^T moving) yields y^T rows 0-63 and the
    softmax denominator in row 64 of the same PSUM tile.
  - Causal mask: for diagonal 128x128 blocks an extra matmul
    (strict-upper-tri x (-1e9 * I)) accumulates -1e9 above the diagonal
    before exp; exp(scale=0.125) on ScalarE evacuates PSUM->SBUF.
  - softmax normalization: 1/s via DVE reciprocal on a partition-spread
    copy of the denominators, broadcast to head-feature rows via a tiny
    K=4 PE matmul, multiplied in by DVE before c_proj.
"""
import numpy as np
from contextlib import ExitStack

import concourse.bass as bass
import concourse.tile as tile
from concourse import bacc, mybir
from concourse.bass_utils import run_bass_kernel_spmd

FP32 = mybir.dt.float32
EXP = mybir.ActivationFunctionType.Exp

T = 2048
C = 1024
NH_CORE = 4          # heads per core
DH = 64
N_CORES = 8

_CACHE = {}


def _build_nc():
    nc = bacc.Bacc("TRN2", target_bir_lowering=False, debug=False,
                   num_devices=N_CORES)

    xt_d = nc.dram_tensor("xt", (128, 8 * T), FP32, kind="ExternalInput")
    wqk_d = nc.dram_tensor("wqk", (128, 8 * 512), FP32, kind="ExternalInput")
    wv_d = nc.dram_tensor("wv", (128, 8 * 256), FP32, kind="ExternalInput")
    wp_d = nc.dram_tensor("wproj", (128, 2 * C), FP32, kind="ExternalInput")
    u_d = nc.dram_tensor("u128", (128, 128), FP32, kind="ExternalInput")
    ni_d = nc.dram_tensor("negi", (128, 128), FP32, kind="ExternalInput")
    eab_d = nc.dram_tensor("eab", (4, 256), FP32, kind="ExternalInput")
    out_d = nc.dram_tensor("out", (T, C), FP32, kind="ExternalOutput")

    with tile.TileContext(nc) as tc, ExitStack() as top:
        const = top.enter_context(tc.tile_pool(name="const", bufs=1))
        persist = top.enter_context(tc.tile_pool(name="persist", bufs=1))
        ppool = top.enter_context(tc.tile_pool(name="ppool", bufs=3))

        es_attn = ExitStack()   # attention psum pools
        es_x = ExitStack()      # x^T + projection weights, freed after attention p0
        ps_S = es_attn.enter_context(
            tc.tile_pool(name="ps_S", bufs=2, space="PSUM"))
        ps_y = es_attn.enter_context(
            tc.tile_pool(name="ps_y", bufs=1, space="PSUM"))
        xpool = es_x.enter_context(tc.tile_pool(name="xpool", bufs=1))
        ps_proj = es_x.enter_context(
            tc.tile_pool(name="ps_proj", bufs=2, space="PSUM"))

        # ---- constants / weights / inputs ----
        u_sb = const.tile([128, 128], FP32, tag="u")
        ni_sb = const.tile([128, 128], FP32, tag="ni")
        eab_sb = const.tile([4, 256], FP32, tag="eab")
        wqk_sb = xpool.tile([128, 8 * 512], FP32, tag="wqk")
        wv_sb = xpool.tile([128, 8 * 256], FP32, tag="wv")
        xt_sb = xpool.tile([128, 8 * T], FP32, tag="xt")

        nc.gpsimd.dma_start(wqk_sb[:], wqk_d.ap())
        nc.gpsimd.dma_start(u_sb[:], u_d.ap())
        nc.gpsimd.dma_start(ni_sb[:], ni_d.ap())
        nc.gpsimd.dma_start(eab_sb[:], eab_d.ap())
        nc.gpsimd.dma_start(wv_sb[:], wv_d.ap())
        for j in range(8):
            nc.gpsimd.dma_start(xt_sb[:, T * j:T * (j + 1)],
                                xt_d.ap()[:, T * j:T * (j + 1)])

        # q^T / k^T chunks: [128, 2048] each, two heads per chunk
        qt0 = persist.tile([128, T], FP32, tag="qt0")
        qt1 = persist.tile([128, T], FP32, tag="qt1")
        kt0 = persist.tile([128, T], FP32, tag="kt0")
        kt1 = persist.tile([128, T], FP32, tag="kt1")
        # V augmented: per token-block t of 128, per head h: 64 V cols + ones
        vaug = persist.tile([128, 16 * 260], FP32, tag="vaug")
        # y^T + denominator per head: [65, 2048]
        yfull = [persist.tile([65, T], FP32, tag=f"yf{h}", name=f"yf{h}")
                 for h in range(4)]

        ones_view = vaug[:].rearrange("p (t h e) -> p (t h) e", t=16, h=4)
        nc.vector.memset(ones_view[:, :, 64:65], 1.0)

        def emit_qk_quarter(m, dst, quarter):
            """qkv^T chunk m, token quarter [512q,512q+512) -> dst slice."""
            pq = ps_proj.tile([128, 512], FP32, tag="proj")
            for c in range(8):
                nc.tensor.matmul(
                    pq[:],
                    wqk_sb[:, 512 * c + 128 * m: 512 * c + 128 * (m + 1)],
                    xt_sb[:, T * c + 512 * quarter: T * c + 512 * (quarter + 1)],
                    start=(c == 0), stop=(c == 7))
            return pq

        # ---- phase 1: q/k chunks for pair 0, then V ----
        for m, dst in ((0, qt0), (2, kt0)):
            for quarter in range(4):
                pq = emit_qk_quarter(m, dst, quarter)
                nc.scalar.copy(dst[:, 512 * quarter: 512 * (quarter + 1)], pq[:])

        for t in range(16):
            pv = ps_proj.tile([128, 256], FP32, tag="proj")
            for c in range(8):
                nc.tensor.matmul(
                    pv[:],
                    xt_sb[:, T * c + 128 * t: T * c + 128 * (t + 1)],
                    wv_sb[:, 256 * c: 256 * (c + 1)],
                    start=(c == 0), stop=(c == 7))
            dst = vaug[:, 260 * t: 260 * (t + 1)].rearrange(
                "p (h e) -> p h e", h=4)[:, :, 0:64]
            src = pv[:].rearrange("p (h e) -> p h e", h=4)
            nc.vector.tensor_copy(dst, src)

        # ---- filler generator: pair-1 q/k chunks, interleaved into attention p0
        def filler_gen():
            for m, dst in ((1, qt1), (3, kt1)):
                for quarter in range(4):
                    pq = ps_proj.tile([128, 512], FP32, tag="proj")
                    for c in range(8):
                        nc.tensor.matmul(
                            pq[:],
                            wqk_sb[:, 512 * c + 128 * m: 512 * c + 128 * (m + 1)],
                            xt_sb[:, T * c + 512 * quarter:
                                  T * c + 512 * (quarter + 1)],
                            start=(c == 0), stop=(c == 7))
                        yield
                    nc.vector.tensor_copy(
                        dst[:, 512 * quarter: 512 * (quarter + 1)], pq[:])
                    yield

        fill = filler_gen()

        def emit_attention(pair, qt, kt):
            for qc in range(4):
                yA = ps_y.tile([65, 512], FP32, tag="yA")
                yB = ps_y.tile([65, 512], FP32, tag="yB")
                njs = 4 * qc + 4
                pend = None
                for j in range(njs):
                    diag = j >= 4 * qc
                    off = 128 * (j - 4 * qc) if diag else 0
                    S = ps_S.tile([128, 1024], FP32, tag="S")
                    qlo = 512 * qc + off
                    qhi = 512 * (qc + 1)
                    nc.tensor.matmul(
                        S[:, off:512],
                        kt[0:64, 128 * j:128 * (j + 1)],
                        qt[0:64, qlo:qhi],
                        start=True, stop=not diag)
                    nc.tensor.matmul(
                        S[:, 512 + off:1024],
                        kt[64:128, 128 * j:128 * (j + 1)],
                        qt[64:128, qlo:qhi],
                        start=True, stop=not diag)
                    if diag:
                        nc.tensor.matmul(S[:, off:off + 128], u_sb[:], ni_sb[:],
                                         start=False, stop=True)
                        nc.tensor.matmul(S[:, 512 + off:512 + off + 128],
                                         u_sb[:], ni_sb[:],
                                         start=False, stop=True)
                    P = ppool.tile([128, 1024], FP32, tag="P")
                    sv = S[:].rearrange("p (a n) -> p a n", a=2)[:, :, off:]
                    pvw = P[:].rearrange("p (a n) -> p a n", a=2)[:, :, off:]
                    nc.scalar.activation(pvw, sv, EXP, scale=0.125)
                    if pair == 0:
                        for _ in range(2):
                            if next(fill, "done") == "done":
                                break
                    if pend is not None:
                        pend()
                    def make_pv(j=j, off=off, P=P, first=(j == 0),
                                last=(j == njs - 1), yA=yA, yB=yB):
                        def emit():
                            nc.tensor.matmul(
                                yA[:, off:512],
                                vaug[:, 260 * j + 65 * (2 * pair):
                                     260 * j + 65 * (2 * pair) + 65],
                                P[:, off:512],
                                start=first, stop=last)
                            nc.tensor.matmul(
                                yB[:, off:512],
                                vaug[:, 260 * j + 65 * (2 * pair + 1):
                                     260 * j + 65 * (2 * pair + 1) + 65],
                                P[:, 512 + off:1024],
                                start=first, stop=last)
                        return emit
                    pend = make_pv()
                pend()
                nc.vector.tensor_copy(
                    yfull[2 * pair][:, 512 * qc: 512 * (qc + 1)], yA[:])
                nc.vector.tensor_copy(
                    yfull[2 * pair + 1][:, 512 * qc: 512 * (qc + 1)], yB[:])

        emit_attention(0, qt0, kt0)
        for _ in fill:     # drain any remaining pair-1 projection work
            pass
        es_x.close()       # frees x^T/wqk/wv SBUF + proj PSUM
        emit_attention(1, qt1, kt1)
        es_attn.close()

        # ---- phase 4: normalization ----
        es_tail = ExitStack()
        tail = es_tail.enter_context(tc.tile_pool(name="tail", bufs=1))
        ps_rb = es_tail.enter_context(
            tc.tile_pool(name="ps_rb", bufs=2, space="PSUM"))
        ps_o = es_tail.enter_context(
            tc.tile_pool(name="ps_o", bufs=2, space="PSUM"))
        ostage = es_tail.enter_context(tc.tile_pool(name="ostage", bufs=3))

        wp_sb = tail.tile([128, 2 * C], FP32, tag="wp")
        nc.gpsimd.dma_start(wp_sb[:], wp_d.ap())

        ypair = [tail.tile([128, T], FP32, tag=f"yp{p}", name=f"yp{p}")
                 for p in range(2)]
        ynorm = [tail.tile([128, T], FP32, tag=f"yn{p}", name=f"yn{p}")
                 for p in range(2)]
        sr = tail.tile([128, 64], FP32, tag="sr")
        rr = tail.tile([128, 64], FP32, tag="rr")
        r4 = tail.tile([4, T], FP32, tag="r4")

        for p in range(2):
            nc.gpsimd.dma_start(ypair[p][0:64, :], yfull[2 * p][0:64, :])
            nc.gpsimd.dma_start(ypair[p][64:128, :], yfull[2 * p + 1][0:64, :])
        for h in range(4):
            nc.gpsimd.dma_start(sr[32 * h:32 * (h + 1), :], yfull[h][64:65, :])
        nc.vector.reciprocal(rr[:], sr[:])
        for h in range(4):
            nc.gpsimd.dma_start(r4[h:h + 1, :], rr[32 * h:32 * (h + 1), :])

        for p in range(2):
            for qc in range(4):
                rb = ps_rb.tile([128, 512], FP32, tag="rb")
                nc.tensor.matmul(rb[:], eab_sb[0:4, 128 * p:128 * (p + 1)],
                                 r4[0:4, 512 * qc:512 * (qc + 1)],
                                 start=True, stop=True)
                nc.vector.tensor_mul(
                    ynorm[p][:, 512 * qc:512 * (qc + 1)],
                    ypair[p][:, 512 * qc:512 * (qc + 1)], rb[:])

        # ---- phase 5: c_proj + output ----
        for t in range(16):
            po = ps_o.tile([128, C], FP32, tag="po")
            for p in range(2):
                for n2 in range(2):
                    nc.tensor.matmul(
                        po[:, 512 * n2:512 * (n2 + 1)],
                        ynorm[p][:, 128 * t:128 * (t + 1)],
                        wp_sb[:, C * p + 512 * n2: C * p + 512 * (n2 + 1)],
                        start=(p == 0), stop=(p == 1))
            ost = ostage.tile([128, C], FP32, tag="ost")
            if t % 2 == 0:
                nc.scalar.copy(ost[:], po[:])
            else:
                nc.vector.tensor_copy(ost[:], po[:])
            nc.gpsimd.dma_start(out_d.ap()[128 * t:128 * (t + 1), :], ost[:])

        es_tail.close()

    nc.compile()
    return nc


def _get_nc():
    if "nc" not in _CACHE:
        _CACHE["nc"] = _build_nc()
    return _CACHE["nc"]


def make_in_maps(x, w_attn, w_proj):
    """Build the 8 per-core input dicts from the full inputs."""
    x = np.asarray(x, np.float32)
    w_attn = np.asarray(w_attn, np.float32)
    w_proj = np.asarray(w_proj, np.float32)

    u128 = np.triu(np.ones((128, 128), np.float32), 1)
    negi = (-1e9 * np.eye(128)).astype(np.float32)
    eab = np.zeros((4, 256), np.float32)
    eab[0, 0:64] = 1.0
    eab[1, 64:128] = 1.0
    eab[2, 128 + 0:128 + 64] = 1.0
    eab[3, 128 + 64:128 + 128] = 1.0

    in_maps = []
    for core in range(N_CORES):
        b, hg = divmod(core, 4)
        x_b = x[b]                                   # [T, C]
        # x^T chunks: [128, 8, T] -> [128, 8T]
        xt = np.ascontiguousarray(
            x_b.reshape(T, 8, 128).transpose(2, 1, 0)).reshape(128, 8 * T)
        wq = w_attn[:, hg * 256:(hg + 1) * 256]
        wk = w_attn[:, C + hg * 256:C + (hg + 1) * 256]
        wqk = np.concatenate([wq, wk], axis=1)        # [1024, 512]
        wqk = np.ascontiguousarray(
            wqk.reshape(8, 128, 512).transpose(1, 0, 2)).reshape(128, 8 * 512)
        wv = w_attn[:, 2 * C + hg * 256:2 * C + (hg + 1) * 256]  # [1024, 256]
        wv = np.ascontiguousarray(
            wv.reshape(8, 128, 256).transpose(1, 0, 2)).reshape(128, 8 * 256)
        wp = w_proj[hg * 256:(hg + 1) * 256, :]       # [256, 1024]
        wp = np.concatenate([wp[0:128, :], wp[128:256, :]],
                            axis=1)                   # [128, 2048]
        in_maps.append({
            "xt": xt, "wqk": wqk, "wv": np.ascontiguousarray(wv),
            "wproj": np.ascontiguousarray(wp),
            "u128": u128, "negi": negi, "eab": eab,
        })
    return in_maps


def gather_out(results):
    """Sum the 4 head-group partials per batch element."""
    out = np.zeros((2, T, C), np.float32)
    for core in range(N_CORES):
        b = core // 4
        out[b] += results[core]["out"]
    return out


def kernel(x, w_attn, w_proj):
    nc = _get_nc()
    in_maps = make_in_maps(x, w_attn, w_proj)
    res = run_bass_kernel_spmd(nc, in_maps, core_ids=list(range(N_CORES)))
    return gather_out(res.results)
